# revision 1
# baseline (speedup 1.0000x reference)
"""Trainium2 Bass kernel for the 2-qubit quantum-circuit batch evaluation.

Reference semantics (per batch row, x = [x0, x1], scalar theta):
    state = RY(theta) @ CNOT @ (RY(x0)|0> ⊗ RY(x1)|0>)
    out = (<Z>, +1)/2 for each qubit.

Algebraically this reduces (product/half-angle identities) to:
    out0 = 0.5 + 0.5*cos(theta)*cos(x0) - 0.5*sin(theta)*sin(x0)*sin(x1)
    out1 = 0.5 + 0.5*cos(x0)*cos(x1)

So the device kernel is a pure streaming map: per element-pair it needs
sin/cos of both angles (ScalarE `Sin` activation; cos via bias=pi/2) and a
handful of elementwise combines (VectorE / ScalarE), making it HBM-bound.

Sharding: pure data parallel over 8 NeuronCores; theta-derived scalars
(0.5*cos(theta), -0.5*sin(theta)) are computed on host and passed as a tiny
replicated [128, 2] constant tensor.
"""

import numpy as np

import concourse.bass as bass
import concourse.mybir as mybir
from concourse.alu_op_type import AluOpType
from concourse.bacc import Bacc
from concourse.tile import TileContext
from concourse import bass_utils

N_CORES = 8
B = 8388608
BC = B // N_CORES            # rows per core
ELEMS = BC * 2               # flat f32 elements per core
P = 128                      # SBUF partitions
F = 4096                     # free elems per partition per tile
T = ELEMS // (P * F)         # tiles per core
HALF_PI = float(np.pi / 2)
MAGIC = float(1.5 * 2**23)   # f32 round-to-nearest-int magic constant

_CACHE = {}


def _build_nc():
    # Bacc (not raw Bass): its compile() pass splits multi-wait sync_info into
    # EventSemaphore instructions — TRN2 allows at most 1 wait per instruction.
    nc = Bacc()
    x = nc.dram_tensor("x", [BC, 2], mybir.dt.float32, kind="ExternalInput")
    consts = nc.dram_tensor("consts", [P, 5], mybir.dt.float32, kind="ExternalInput")
    out = nc.dram_tensor("out", [BC, 2], mybir.dt.float32, kind="ExternalOutput")

    x_t = x[:].flatten().rearrange("(n p f) -> n p f", p=P, f=F)
    o_t = out[:].flatten().rearrange("(n p f) -> n p f", p=P, f=F)

    f32 = mybir.dt.float32
    Sin = mybir.ActivationFunctionType.Sin
    Ident = mybir.ActivationFunctionType.Identity

    with TileContext(nc) as tc:
        with tc.tile_pool(name="cpool", bufs=1) as cpool, \
             tc.tile_pool(name="io", bufs=2) as io, \
             tc.tile_pool(name="work", bufs=2) as work:
            ct = cpool.tile([P, 5], f32)
            nc.sync.dma_start(out=ct[:], in_=consts[:])
            hc = ct[:, 0:1]      # 0.5*cos(theta)
            ns = ct[:, 1:2]      # -0.5*sin(theta)
            half = ct[:, 2:3]    # 0.5
            halfpi = ct[:, 3:4]  # pi/2
            negpi = ct[:, 4:5]   # -pi

            for i in range(T):
                xt = io.tile([P, F], f32, tag="xt")
                nc.sync.dma_start(out=xt[:], in_=x_t[i])

                # Range reduction: ACT Sin is only accurate for |arg| <= pi,
                # but x spans ~±17. Magic-number rounding (mod isn't valid DVE
                # ISA): t = x/(2pi) + 1.5*2^23 forces round-to-nearest-int in
                # the mantissa; k2 = (t - MAGIC)*(-2pi) = -2pi*round(x/2pi);
                # y = x + k2 in [-pi, pi]. sin(x) = Sin(y); cos by evenness:
                # cos(x) = Sin(pi/2 - |y|), abs split across ACT/DVE to balance.
                t = work.tile([P, F], f32, tag="t")
                y = work.tile([P, F], f32, tag="y")
                nc.vector.tensor_scalar(
                    t[:], xt[:], float(1.0 / (2 * np.pi)), MAGIC,
                    AluOpType.mult, AluOpType.add,
                )
                # k2 in place of t, then y = x + k2
                nc.vector.tensor_scalar(
                    t[:], t[:], MAGIC, float(-2 * np.pi),
                    AluOpType.subtract, AluOpType.mult,
                )
                nc.vector.tensor_tensor(y[:], xt[:], t[:], AluOpType.add)
                # S reuses t's slots (t is dead after y)
                S = work.tile([P, F], f32, tag="t")
                nc.scalar.activation(S[:], y[:], Sin)
                # |y| then C = Sin(pi/2 - |y|), both in place of y
                nc.scalar.activation(y[:], y[:], mybir.ActivationFunctionType.Abs)
                C = y
                nc.scalar.activation(C[:], y[:], Sin, bias=halfpi, scale=-1.0)

                Sv = S[:].rearrange("p (k two) -> p k two", two=2)
                Cv = C[:].rearrange("p (k two) -> p k two", two=2)
                o = io.tile([P, F], f32, tag="o")
                ov = o[:].rearrange("p (k two) -> p k two", two=2)

                m = work.tile([P, F // 2], f32, tag="m")
                g = work.tile([P, F // 2], f32, tag="g")
                a = work.tile([P, F // 2], f32, tag="a")
                m2 = m  # m2 = Copy(m*ns) in place

                # m = sin(x0)*sin(x1); g = cos(x0)*cos(x1)
                nc.vector.tensor_tensor(m[:], Sv[:, :, 0], Sv[:, :, 1], AluOpType.mult)
                nc.vector.tensor_tensor(g[:], Cv[:, :, 0], Cv[:, :, 1], AluOpType.mult)
                # a = 0.5*cos(theta)*cos(x0) + 0.5   (ScalarE, runtime scale)
                nc.scalar.activation(a[:], Cv[:, :, 0], Ident, bias=half, scale=hc)
                # m2 = -0.5*sin(theta)*m (ACT Copy, runtime scale);
                # out1 = 0.5*g + 0.5 (ACT); out0 = a + m2 (DVE).
                Copy = mybir.ActivationFunctionType.Copy
                nc.scalar.activation(m2[:], m[:], Copy, scale=ns)
                nc.scalar.activation(ov[:, :, 1], g[:], Ident, bias=half, scale=half)
                nc.vector.tensor_tensor(ov[:, :, 0], a[:], m2[:], AluOpType.add)

                nc.sync.dma_start(out=o_t[i], in_=o[:])
    nc.compile()
    return nc


def _run(in_maps, trace=False, trace_cores=None):
    if "nc" not in _CACHE:
        _CACHE["nc"] = _build_nc()
    return bass_utils.run_bass_kernel_spmd(
        _CACHE["nc"],
        in_maps,
        core_ids=list(range(N_CORES)),
        trace=trace,
        trace_cores=trace_cores,
    )


def kernel(x, theta, _trace=False, _trace_cores=None):
    x = np.ascontiguousarray(np.asarray(x, dtype=np.float32))
    theta = np.asarray(theta, dtype=np.float32)
    assert x.shape == (B, 2), x.shape

    th = float(theta.reshape(-1)[0])
    consts = np.empty((P, 5), dtype=np.float32)
    consts[:, 0] = 0.5 * np.cos(th)
    consts[:, 1] = -0.5 * np.sin(th)
    consts[:, 2] = 0.5
    consts[:, 3] = HALF_PI
    consts[:, 4] = -np.pi

    shards = x.reshape(N_CORES, BC, 2)
    in_maps = [{"x": shards[c], "consts": consts} for c in range(N_CORES)]

    res = _run(in_maps, trace=_trace, trace_cores=_trace_cores)
    _CACHE["last_results"] = res
    out = np.concatenate([res.results[c]["out"] for c in range(N_CORES)], axis=0)
    return out

